# revision 23
# baseline (speedup 1.0000x reference)
"""BitLinear (per-token int8 activation quant + ternary weight quant + matmul)
as a Bass/Tile kernel on 8 Trainium2 NeuronCores.

Strategy (data-parallel tokens + 8-way-sharded |W| mean + rotated slabs):
  - x [4,2048,4096] -> [8192,4096]; each core quantizes and matmuls its own
    1024-token slab against the FULL weight; outputs concatenate on tokens.
  - mean(|W|) is sharded: core c reads only its own 512-of-row slice of W
    (8.4 MB), computes a partial |W| sum on the GPSIMD engine (keeping the
    Vector engine free for activation quant), and a 512-byte AllReduce(add)
    yields the global sum.
  - W is passed HOST-TRANSPOSED ([in, of] layout) so ternarization produces
    matmul-ready [contraction, of] tiles directly -- no SBUF-to-SBUF xbar
    transposes in the matmul phase (they contended with PE operand streams
    and serialized behind the collective's DMA-completion lane).
  - Each core walks the 8 output slabs in ROTATED order starting with its
    own slice (first chunks prefetched during the AllReduce wait),
    ternarizing one slab ahead of the PE, interleaved per token tile so the
    Vector FIFO never head-of-line blocks the PSUM-drain scales.
  - q = rint(x*s) (s = 127/max(|x|) per token) and tw in {-1,0,1} are exact
    in bf16 => the bf16 matmul with fp32 PSUM accumulation is EXACT integer
    arithmetic; per-token dequant scales applied on the PSUM->SBUF copy.
"""
import numpy as np
from contextlib import ExitStack

N_CORES = 8
B, S, D_IN, D_OUT = 4, 2048, 4096, 4096
TOK = B * S                  # 8192
TOK_PC = TOK // N_CORES      # 1024 tokens per core
N_TOK_TILES = TOK_PC // 128  # 8
N_K = D_IN // 128            # 32 contraction tiles
OF_CHUNK = 512
N_SLAB = D_OUT // OF_CHUNK   # 8
NCH = 8                      # ternarize chunks per slab: [128, 4, 512] each
KCH = N_K // NCH             # k-tiles per chunk (4)
EPS = 1e-5
MAGIC = float(np.float32(1.5 * 2 ** 23))   # fp32 round-to-nearest-even trick
MEAN_SCALE = float(np.float32(1.0 / (D_IN * D_OUT)))  # 2^-24, exact

_CACHE = {}


def _build_module():
    import concourse.bacc as bacc
    import concourse.tile as tile
    import concourse.mybir as mybir
    import concourse.bass_isa as bass_isa

    dt = mybir.dt
    AF = mybir.ActivationFunctionType
    AL = mybir.AluOpType
    AX = mybir.AxisListType

    nc = bacc.Bacc(
        "TRN2", target_bir_lowering=False, debug=False, num_devices=N_CORES
    )
    xs = nc.dram_tensor("xs", [TOK_PC, D_IN], dt.float32, kind="ExternalInput").ap()
    # own W slice, row-major [of, in]: mean-pass reads (8 KB DMA lines)
    wmy = nc.dram_tensor(
        "wmy", [OF_CHUNK, D_IN], dt.float32, kind="ExternalInput"
    ).ap()
    # own W slice, transposed: [in, of] for out-feature rows [512c, 512c+512)
    wmyT = nc.dram_tensor(
        "wmyT", [D_IN, OF_CHUNK], dt.float32, kind="ExternalInput"
    ).ap()
    # remaining 7 slabs, transposed + rotated: of-cols of slab c+1, ... (mod 8)
    wrotT = nc.dram_tensor(
        "wrotT", [D_IN, (N_SLAB - 1) * OF_CHUNK], dt.float32, kind="ExternalInput"
    ).ap()
    out = nc.dram_tensor("out", [TOK_PC, D_OUT], dt.float32, kind="ExternalOutput").ap()

    HD = D_IN // 2
    HK = N_K // 2
    CH_F = KCH * OF_CHUNK    # 2048 free elems per ternarize chunk

    def wT_chunk(src, j, col0):
        # [128, KCH, OF_CHUNK] view of rows [j*512,(j+1)*512), cols [col0,+512)
        return src[j * 512:(j + 1) * 512, col0:col0 + OF_CHUNK].rearrange(
            "(a p) f -> p a f", p=128
        )

    with tile.TileContext(nc) as tc, ExitStack() as ctx:
        stats = ctx.enter_context(tc.tile_pool(name="stats", bufs=1))
        qT_pool = ctx.enter_context(tc.tile_pool(name="qT", bufs=N_TOK_TILES))
        big = ctx.enter_context(tc.tile_pool(name="big", bufs=3))
        qb_pool = ctx.enter_context(tc.tile_pool(name="qbp", bufs=2))
        twTp = ctx.enter_context(tc.tile_pool(name="twT", bufs=2))
        op = ctx.enter_context(tc.tile_pool(name="op", bufs=3))
        pp = ctx.enter_context(tc.tile_pool(name="pp", bufs=8, space="PSUM"))
        dram = ctx.enter_context(tc.tile_pool(name="dram", bufs=1, space="DRAM"))

        amc = stats.tile([128, N_TOK_TILES], dt.float32, tag="amc")
        s_all = stats.tile([128, N_TOK_TILES], dt.float32, tag="s_all")
        dq = stats.tile([128, N_TOK_TILES], dt.float32, tag="dq")
        wme = stats.tile([128, 1], dt.float32, tag="wme")
        swt = stats.tile([128, 1], dt.float32, tag="swt")
        wp = stats.tile([128, NCH], dt.float32, tag="wp")
        wsum_sb = stats.tile([128, 1], dt.float32, tag="wsum_sb")
        gtot = stats.tile([128, 1], dt.float32, tag="gtot")
        gsum = stats.tile([128, 1], dt.float32, tag="gsum")
        tmp1 = stats.tile([128, 1], dt.float32, tag="tmp1")

        inb = dram.tile([128, 1], dt.float32, tag="inb")
        outb = dram.tile([128, 1], dt.float32, tag="outb")

        # ---- |W| partial sum over own slice (row-major reads: contiguous
        # 8 KB DMA lines). Reduces run on the Scalar engine via activation
        # accumulate (Abs + accum_out), keeping Vector free for x-quant;
        # high priority so the AllReduce trigger fires asap. ----
        with nc.named_scope("wmean"), tc.high_priority():
            for j in range(NCH):
                wt = big.tile([128, CH_F], dt.float32, tag="big", name=f"wm{j}")
                eng = nc.scalar if j % 2 == 0 else nc.sync
                eng.dma_start(
                    wt[:],
                    wmy[(j // 2) * 128:(j // 2 + 1) * 128,
                        (j % 2) * CH_F:(j % 2 + 1) * CH_F],
                )
                nc.scalar.activation(
                    wt[:], wt[:], AF.Abs, accum_out=wp[:, j:j + 1]
                )
            nc.vector.tensor_reduce(wsum_sb[:], wp[:], axis=AX.X, op=AL.add)

        # ---- x-quant: own tokens -> resident qT tiles (half tiles) ----
        # software-pipelined: tile t's scale/round/transpose are emitted
        # after tile t+1's amax chain, so the Vector FIFO never stalls
        # waiting for the Scalar engine's scale pass.
        qT_tiles = []
        with nc.named_scope("xquant"), tc.tile_pool(name="xq", bufs=4) as xq:

            def xq_flush(t, xh, qT_t):
                for h in range(2):
                    nc.scalar.activation(
                        xh[h][:], xh[h][:], AF.Copy, scale=s_all[:, t:t + 1]
                    )
                    qbh = qb_pool.tile(
                        [128, HD], dt.bfloat16, tag="qb", name=f"qb{t}_{h}"
                    )
                    nc.vector.tensor_scalar(
                        qbh[:], xh[h][:], MAGIC, MAGIC, op0=AL.add, op1=AL.subtract
                    )
                    nc.sync.dma_start(
                        qT_t[:, h * HK:(h + 1) * HK, :], qbh[:], transpose=True
                    )

            pend = None
            for t in range(N_TOK_TILES):
                qT_t = qT_pool.tile(
                    [128, N_K, 128], dt.bfloat16, tag="qT", name=f"qT{t}"
                )
                xh = []
                for h in range(2):
                    xth = xq.tile([128, HD], dt.float32, tag="xq", name=f"xt{t}_{h}")
                    eng = nc.sync if h == 0 else nc.scalar
                    eng.dma_start(
                        xth[:], xs[t * 128:(t + 1) * 128, h * HD:(h + 1) * HD]
                    )
                    nc.vector.tensor_reduce(
                        amc[:, t:t + 1] if h == 0 else tmp1[:],
                        xth[:], axis=AX.X, op=AL.max, apply_absolute_value=True,
                    )
                    xh.append(xth)
                # amax = max(half0, half1); then clip, s = 127/amax_c
                nc.vector.tensor_tensor(
                    amc[:, t:t + 1], amc[:, t:t + 1], tmp1[:], op=AL.max
                )
                nc.vector.tensor_scalar(
                    amc[:, t:t + 1], amc[:, t:t + 1], EPS, None, op0=AL.max
                )
                nc.vector.reciprocal(s_all[:, t:t + 1], amc[:, t:t + 1])
                nc.vector.tensor_scalar(
                    s_all[:, t:t + 1], s_all[:, t:t + 1], 127.0, None, op0=AL.mult
                )
                if pend is not None:
                    xq_flush(*pend)
                pend = (t, xh, qT_t)
                qT_tiles.append(qT_t)
            xq_flush(*pend)

        # ---- prefetch own-slab chunks during the AllReduce wait (scalar/
        # sync queues; gpsimd stays clear so the AR chain fires asap) ----
        own_pref = []
        for m in range(4):
            wt = big.tile([128, CH_F], dt.float32, tag="big", name=f"wo{m}")
            eng = nc.scalar if m % 2 == 0 else nc.sync
            eng.dma_start(wt[:], wT_chunk(wmyT, m, 0))
            own_pref.append(wt)

        # ---- tiny AllReduce of the partial |W| sums (gpsimd queue).
        # Emitted after x-quant so no x-quant DMA shares a completion lane
        # slot behind the collective. ----
        with nc.named_scope("ar"):
            nc.gpsimd.partition_all_reduce(
                gtot[:], wsum_sb[:], channels=128,
                reduce_op=bass_isa.ReduceOp.add,
            )
            nc.gpsimd.dma_start(inb[:], gtot[:])
            nc.gpsimd.collective_compute(
                "AllReduce",
                mybir.AluOpType.add,
                replica_groups=[list(range(N_CORES))],
                ins=[inb.opt()],
                outs=[outb.opt()],
            )
            nc.gpsimd.dma_start(gsum[:], outb[:])
            nc.vector.tensor_scalar(
                wme[:], gsum[:], MEAN_SCALE, EPS, op0=AL.mult, op1=AL.max
            )
            nc.vector.reciprocal(swt[:], wme[:])
            for t in range(N_TOK_TILES):
                nc.vector.tensor_scalar(
                    dq[:, t:t + 1], amc[:, t:t + 1], wme[:, 0:1],
                    float(np.float32(1.0 / 127.0)), op0=AL.mult, op1=AL.mult,
                )

        # ---- per-slab ternarize, chunk [128, 2048] = 4 k-tiles x 512 of.
        # W arrives pre-transposed, so round/clamp write matmul-ready tiles;
        # no transposes. Round to a twT slice, clamp in place. ----
        def tern_chunk(wt, twT_c, j, tag):
            nc.scalar.activation(wt[:], wt[:], AF.Copy, scale=swt[:, 0:1])
            dst = twT_c[:, j * CH_F:(j + 1) * CH_F]
            nc.vector.tensor_scalar(
                dst, wt[:], MAGIC, MAGIC, op0=AL.add, op1=AL.subtract
            )
            nc.vector.tensor_scalar(
                dst, dst, 1.0, -1.0, op0=AL.min, op1=AL.max
            )

        def tern_own():
            twT_c = twTp.tile(
                [128, N_K * OF_CHUNK], dt.bfloat16, tag="twT", name="twT_own"
            )
            for m in range(NCH):
                if m < len(own_pref):
                    wt = own_pref[m]
                else:
                    wt = big.tile([128, CH_F], dt.float32, tag="big", name=f"wo{m}")
                    eng = nc.scalar if m % 2 == 0 else nc.sync
                    eng.dma_start(wt[:], wT_chunk(wmyT, m, 0))
                tern_chunk(wt, twT_c, m, f"o{m}")
            return twT_c

        # next slab's ternarize is emitted interleaved with this slab's
        # token tiles so the Vector FIFO alternates tern ops with PSUM-drain
        # scales (psum banks free promptly; no head-of-line stall).
        def make_tern_stream(c):
            twT_c = twTp.tile(
                [128, N_K * OF_CHUNK], dt.bfloat16, tag="twT", name=f"twT{c}"
            )
            col0 = (c - 1) * OF_CHUNK
            wts = {}

            def load(m):
                wt = big.tile([128, CH_F], dt.float32, tag="big", name=f"wt{c}_{m}")
                eng = nc.scalar if m % 2 == 0 else nc.sync
                eng.dma_start(wt[:], wT_chunk(wrotT, m, col0))
                wts[m] = wt

            def compute(m):
                tern_chunk(wts.pop(m), twT_c, m, f"{c}_{m}")

            return twT_c, load, compute

        with nc.named_scope("mm"):
            twT_cur = tern_own()
            for c in range(N_SLAB):
                if c + 1 < N_SLAB:
                    twT_next, tern_load, tern_compute = make_tern_stream(c + 1)
                    tern_load(0)
                    tern_load(1)
                else:
                    twT_next = tern_load = tern_compute = None
                for t in range(N_TOK_TILES):
                    if tern_load is not None:
                        if t + 2 < NCH:
                            tern_load(t + 2)
                        tern_compute(t)
                    ps = pp.tile(
                        [128, OF_CHUNK], dt.float32, tag="ps", name=f"ps{c}_{t}"
                    )
                    for k in range(N_K):
                        nc.tensor.matmul(
                            ps[:], qT_tiles[t][:, k, :],
                            twT_cur[:, k * OF_CHUNK:(k + 1) * OF_CHUNK],
                            start=(k == 0), stop=(k == N_K - 1),
                        )
                    ot = op.tile(
                        [128, OF_CHUNK], dt.float32, tag="ot", name=f"ot{c}_{t}"
                    )
                    nc.scalar.activation(
                        ot[:], ps[:], AF.Copy, scale=dq[:, t:t + 1]
                    )
                    nc.gpsimd.dma_start(
                        out[t * 128:(t + 1) * 128,
                            c * OF_CHUNK:(c + 1) * OF_CHUNK],
                        ot[:],
                    )
                twT_cur = twT_next

    nc.compile()
    return nc


def _get_module():
    if "nc" not in _CACHE:
        _CACHE["nc"] = _build_module()
    return _CACHE["nc"]


def _make_in_maps(x2, w2):
    w2T = np.ascontiguousarray(w2.T)  # [in, of]
    maps = []
    for c in range(N_CORES):
        wrotT = np.concatenate(
            [
                w2T[:, ((c + 1 + j) % N_SLAB) * OF_CHUNK:
                    (((c + 1 + j) % N_SLAB) + 1) * OF_CHUNK]
                for j in range(N_SLAB - 1)
            ],
            axis=1,
        )
        maps.append(
            {
                "xs": x2[c * TOK_PC:(c + 1) * TOK_PC],
                "wmy": np.ascontiguousarray(
                    w2[c * OF_CHUNK:(c + 1) * OF_CHUNK]
                ),
                "wmyT": np.ascontiguousarray(
                    w2T[:, c * OF_CHUNK:(c + 1) * OF_CHUNK]
                ),
                "wrotT": np.ascontiguousarray(wrotT),
            }
        )
    return maps


def kernel(x: np.ndarray, weight: np.ndarray) -> np.ndarray:
    from concourse.bass_utils import run_bass_kernel_spmd

    x = np.asarray(x, dtype=np.float32)
    weight = np.asarray(weight, dtype=np.float32)
    x2 = np.ascontiguousarray(x.reshape(TOK, D_IN))
    w2 = np.ascontiguousarray(weight)

    in_maps = _make_in_maps(x2, w2)
    nc = _get_module()
    res = run_bass_kernel_spmd(nc, in_maps, list(range(N_CORES)))
    out_full = np.empty((TOK, D_OUT), dtype=np.float32)
    for c in range(N_CORES):
        oc = res.results[c]["out"]
        for j in range(N_SLAB):
            s = (c + j) % N_SLAB
            out_full[c * TOK_PC:(c + 1) * TOK_PC,
                     s * OF_CHUNK:(s + 1) * OF_CHUNK] = \
                oc[:, j * OF_CHUNK:(j + 1) * OF_CHUNK]
    return out_full.reshape(B, S, D_OUT)


# revision 25
# speedup vs baseline: 1.1211x; 1.1211x over previous
"""BitLinear (per-token int8 activation quant + ternary weight quant + matmul)
as a Bass/Tile kernel on 8 Trainium2 NeuronCores.

Strategy (data-parallel tokens + 8-way-sharded |W| mean + rotated slabs):
  - x [4,2048,4096] -> [8192,4096]; each core quantizes and matmuls its own
    1024-token slab against the FULL weight; outputs concatenate on tokens.
  - mean(|W|) is sharded: core c reads only its own 512-of-row slice of W
    (8.4 MB), computes a partial |W| sum on the GPSIMD engine (keeping the
    Vector engine free for activation quant), and a 512-byte AllReduce(add)
    yields the global sum.
  - W is passed HOST-TRANSPOSED ([in, of] layout) so ternarization produces
    matmul-ready [contraction, of] tiles directly -- no SBUF-to-SBUF xbar
    transposes in the matmul phase (they contended with PE operand streams
    and serialized behind the collective's DMA-completion lane).
  - Each core walks the 8 output slabs in ROTATED order starting with its
    own slice (first chunks prefetched during the AllReduce wait),
    ternarizing one slab ahead of the PE, interleaved per token tile so the
    Vector FIFO never head-of-line blocks the PSUM-drain scales.
  - q = rint(x*s) (s = 127/max(|x|) per token) and tw in {-1,0,1} are exact
    in bf16 => the bf16 matmul with fp32 PSUM accumulation is EXACT integer
    arithmetic; per-token dequant scales applied on the PSUM->SBUF copy.
"""
import numpy as np
from contextlib import ExitStack

N_CORES = 8
B, S, D_IN, D_OUT = 4, 2048, 4096, 4096
TOK = B * S                  # 8192
TOK_PC = TOK // N_CORES      # 1024 tokens per core
N_TOK_TILES = TOK_PC // 128  # 8
N_K = D_IN // 128            # 32 contraction tiles
OF_CHUNK = 512
N_SLAB = D_OUT // OF_CHUNK   # 8
NCH = 8                      # ternarize chunks per slab: [128, 4, 512] each
KCH = N_K // NCH             # k-tiles per chunk (4)
EPS = 1e-5
MAGIC = float(np.float32(1.5 * 2 ** 23))   # fp32 round-to-nearest-even trick
MEAN_SCALE = float(np.float32(1.0 / (D_IN * D_OUT)))  # 2^-24, exact

_CACHE = {}


def _build_module():
    import concourse.bacc as bacc
    import concourse.tile as tile
    import concourse.mybir as mybir
    import concourse.bass_isa as bass_isa

    dt = mybir.dt
    AF = mybir.ActivationFunctionType
    AL = mybir.AluOpType
    AX = mybir.AxisListType

    nc = bacc.Bacc(
        "TRN2", target_bir_lowering=False, debug=False, num_devices=N_CORES
    )
    xs = nc.dram_tensor("xs", [TOK_PC, D_IN], dt.float32, kind="ExternalInput").ap()
    # own W slice, row-major [of, in]: mean-pass reads (8 KB DMA lines)
    wmy = nc.dram_tensor(
        "wmy", [OF_CHUNK, D_IN], dt.float32, kind="ExternalInput"
    ).ap()
    # own W slice, transposed: [in, of] for out-feature rows [512c, 512c+512)
    wmyT = nc.dram_tensor(
        "wmyT", [D_IN, OF_CHUNK], dt.float32, kind="ExternalInput"
    ).ap()
    # remaining 7 slabs, transposed + rotated: of-cols of slab c+1, ... (mod 8)
    wrotT = nc.dram_tensor(
        "wrotT", [D_IN, (N_SLAB - 1) * OF_CHUNK], dt.float32, kind="ExternalInput"
    ).ap()
    out = nc.dram_tensor("out", [TOK_PC, D_OUT], dt.float32, kind="ExternalOutput").ap()

    HD = D_IN // 2
    HK = N_K // 2
    CH_F = KCH * OF_CHUNK    # 2048 free elems per ternarize chunk

    def wT_chunk(src, j, col0):
        # [128, KCH, OF_CHUNK] view of rows [j*512,(j+1)*512), cols [col0,+512)
        return src[j * 512:(j + 1) * 512, col0:col0 + OF_CHUNK].rearrange(
            "(a p) f -> p a f", p=128
        )

    with tile.TileContext(nc) as tc, ExitStack() as ctx:
        stats = ctx.enter_context(tc.tile_pool(name="stats", bufs=1))
        qT_pool = ctx.enter_context(tc.tile_pool(name="qT", bufs=N_TOK_TILES))
        big = ctx.enter_context(tc.tile_pool(name="big", bufs=3))
        qb_pool = ctx.enter_context(tc.tile_pool(name="qbp", bufs=2))
        twTp = ctx.enter_context(tc.tile_pool(name="twT", bufs=2))
        op = ctx.enter_context(tc.tile_pool(name="op", bufs=3))
        pp = ctx.enter_context(tc.tile_pool(name="pp", bufs=8, space="PSUM"))
        dram = ctx.enter_context(tc.tile_pool(name="dram", bufs=1, space="DRAM"))

        amc = stats.tile([128, N_TOK_TILES], dt.float32, tag="amc")
        s_all = stats.tile([128, N_TOK_TILES], dt.float32, tag="s_all")
        dq = stats.tile([128, N_TOK_TILES], dt.float32, tag="dq")
        wme = stats.tile([128, 1], dt.float32, tag="wme")
        swt = stats.tile([128, 1], dt.float32, tag="swt")
        wp = stats.tile([128, NCH], dt.float32, tag="wp")
        wsum_sb = stats.tile([128, 1], dt.float32, tag="wsum_sb")
        gtot = stats.tile([128, 1], dt.float32, tag="gtot")
        gsum = stats.tile([128, 1], dt.float32, tag="gsum")
        tmp1 = stats.tile([128, 1], dt.float32, tag="tmp1")

        inb = dram.tile([128, 1], dt.float32, tag="inb")
        outb = dram.tile([128, 1], dt.float32, tag="outb")

        # ---- |W| partial sum over own slice (row-major reads: contiguous
        # 8 KB DMA lines). Reduces run on the Scalar engine via activation
        # accumulate (Abs + accum_out), keeping Vector free for x-quant;
        # high priority so the AllReduce trigger fires asap. ----
        with nc.named_scope("wmean"), tc.high_priority():
            for j in range(NCH):
                wt = big.tile([128, CH_F], dt.float32, tag="big", name=f"wm{j}")
                eng = nc.scalar if j % 2 == 0 else nc.sync
                eng.dma_start(
                    wt[:],
                    wmy[(j // 2) * 128:(j // 2 + 1) * 128,
                        (j % 2) * CH_F:(j % 2 + 1) * CH_F],
                )
                nc.scalar.activation(
                    wt[:], wt[:], AF.Abs, accum_out=wp[:, j:j + 1]
                )
            nc.vector.tensor_reduce(wsum_sb[:], wp[:], axis=AX.X, op=AL.add)

        # ---- x-quant: own tokens -> resident qT tiles (half tiles) ----
        # software-pipelined: tile t's scale/round/transpose are emitted
        # after tile t+1's amax chain, so the Vector FIFO never stalls
        # waiting for the Scalar engine's scale pass. High priority places
        # the qT transposes BEFORE the collective in the static schedule:
        # DMA-transposes scheduled after an in-flight collective stall
        # until it completes (observed consistently), so they must come
        # first; the collective still executes early since its gpsimd
        # queue slot has no work ahead of it.
        qT_tiles = []
        with nc.named_scope("xquant"), tc.high_priority(), \
                tc.tile_pool(name="xq", bufs=4) as xq:

            def xq_flush(t, xh, qT_t):
                for h in range(2):
                    nc.scalar.activation(
                        xh[h][:], xh[h][:], AF.Copy, scale=s_all[:, t:t + 1]
                    )
                    qbh = qb_pool.tile(
                        [128, HD], dt.bfloat16, tag="qb", name=f"qb{t}_{h}"
                    )
                    nc.vector.tensor_scalar(
                        qbh[:], xh[h][:], MAGIC, MAGIC, op0=AL.add, op1=AL.subtract
                    )
                    nc.sync.dma_start(
                        qT_t[:, h * HK:(h + 1) * HK, :], qbh[:], transpose=True
                    )

            pend = None
            for t in range(N_TOK_TILES):
                qT_t = qT_pool.tile(
                    [128, N_K, 128], dt.bfloat16, tag="qT", name=f"qT{t}"
                )
                xh = []
                for h in range(2):
                    xth = xq.tile([128, HD], dt.float32, tag="xq", name=f"xt{t}_{h}")
                    eng = nc.sync if h == 0 else nc.scalar
                    eng.dma_start(
                        xth[:], xs[t * 128:(t + 1) * 128, h * HD:(h + 1) * HD]
                    )
                    nc.vector.tensor_reduce(
                        amc[:, t:t + 1] if h == 0 else tmp1[:],
                        xth[:], axis=AX.X, op=AL.max, apply_absolute_value=True,
                    )
                    xh.append(xth)
                # amax = max(half0, half1); then clip, s = 127/amax_c
                nc.vector.tensor_tensor(
                    amc[:, t:t + 1], amc[:, t:t + 1], tmp1[:], op=AL.max
                )
                nc.vector.tensor_scalar(
                    amc[:, t:t + 1], amc[:, t:t + 1], EPS, None, op0=AL.max
                )
                nc.vector.reciprocal(s_all[:, t:t + 1], amc[:, t:t + 1])
                nc.vector.tensor_scalar(
                    s_all[:, t:t + 1], s_all[:, t:t + 1], 127.0, None, op0=AL.mult
                )
                if pend is not None:
                    xq_flush(*pend)
                pend = (t, xh, qT_t)
                qT_tiles.append(qT_t)
            xq_flush(*pend)

        # ---- prefetch own-slab chunks during the AllReduce wait (scalar/
        # sync queues; gpsimd stays clear so the AR chain fires asap) ----
        own_pref = []
        for m in range(4):
            wt = big.tile([128, CH_F], dt.float32, tag="big", name=f"wo{m}")
            eng = nc.scalar if m % 2 == 0 else nc.sync
            eng.dma_start(wt[:], wT_chunk(wmyT, m, 0))
            own_pref.append(wt)

        # ---- tiny AllReduce of the partial |W| sums (gpsimd queue).
        # Emitted after x-quant so no x-quant DMA shares a completion lane
        # slot behind the collective. ----
        with nc.named_scope("ar"):
            with tc.high_priority():
                nc.gpsimd.partition_all_reduce(
                    gtot[:], wsum_sb[:], channels=128,
                    reduce_op=bass_isa.ReduceOp.add,
                )
                nc.gpsimd.dma_start(inb[:], gtot[:])
                nc.gpsimd.collective_compute(
                    "AllReduce",
                    mybir.AluOpType.add,
                    replica_groups=[list(range(N_CORES))],
                    ins=[inb.opt()],
                    outs=[outb.opt()],
                )
                nc.gpsimd.dma_start(gsum[:], outb[:])
            nc.vector.tensor_scalar(
                wme[:], gsum[:], MEAN_SCALE, EPS, op0=AL.mult, op1=AL.max
            )
            nc.vector.reciprocal(swt[:], wme[:])
            for t in range(N_TOK_TILES):
                nc.vector.tensor_scalar(
                    dq[:, t:t + 1], amc[:, t:t + 1], wme[:, 0:1],
                    float(np.float32(1.0 / 127.0)), op0=AL.mult, op1=AL.mult,
                )

        # ---- per-slab ternarize, chunk [128, 2048] = 4 k-tiles x 512 of.
        # W arrives pre-transposed, so round/clamp write matmul-ready tiles;
        # no transposes. Round to a twT slice, clamp in place. ----
        def tern_chunk(wt, twT_c, j, tag):
            nc.scalar.activation(wt[:], wt[:], AF.Copy, scale=swt[:, 0:1])
            dst = twT_c[:, j * CH_F:(j + 1) * CH_F]
            nc.vector.tensor_scalar(
                dst, wt[:], MAGIC, MAGIC, op0=AL.add, op1=AL.subtract
            )
            nc.vector.tensor_scalar(
                dst, dst, 1.0, -1.0, op0=AL.min, op1=AL.max
            )

        def tern_own():
            twT_c = twTp.tile(
                [128, N_K * OF_CHUNK], dt.bfloat16, tag="twT", name="twT_own"
            )
            for m in range(NCH):
                if m < len(own_pref):
                    wt = own_pref[m]
                else:
                    wt = big.tile([128, CH_F], dt.float32, tag="big", name=f"wo{m}")
                    eng = nc.scalar if m % 2 == 0 else nc.sync
                    eng.dma_start(wt[:], wT_chunk(wmyT, m, 0))
                tern_chunk(wt, twT_c, m, f"o{m}")
            return twT_c

        # next slab's ternarize is emitted interleaved with this slab's
        # token tiles so the Vector FIFO alternates tern ops with PSUM-drain
        # scales (psum banks free promptly; no head-of-line stall).
        def make_tern_stream(c):
            twT_c = twTp.tile(
                [128, N_K * OF_CHUNK], dt.bfloat16, tag="twT", name=f"twT{c}"
            )
            col0 = (c - 1) * OF_CHUNK
            wts = {}

            def load(m):
                wt = big.tile([128, CH_F], dt.float32, tag="big", name=f"wt{c}_{m}")
                eng = nc.scalar if m % 2 == 0 else nc.sync
                eng.dma_start(wt[:], wT_chunk(wrotT, m, col0))
                wts[m] = wt

            def compute(m):
                tern_chunk(wts.pop(m), twT_c, m, f"{c}_{m}")

            return twT_c, load, compute

        with nc.named_scope("mm"):
            twT_cur = tern_own()
            for c in range(N_SLAB):
                if c + 1 < N_SLAB:
                    twT_next, tern_load, tern_compute = make_tern_stream(c + 1)
                    tern_load(0)
                    tern_load(1)
                else:
                    twT_next = tern_load = tern_compute = None
                for t in range(N_TOK_TILES):
                    if tern_load is not None:
                        if t + 2 < NCH:
                            tern_load(t + 2)
                        tern_compute(t)
                    ps = pp.tile(
                        [128, OF_CHUNK], dt.float32, tag="ps", name=f"ps{c}_{t}"
                    )
                    for k in range(N_K):
                        nc.tensor.matmul(
                            ps[:], qT_tiles[t][:, k, :],
                            twT_cur[:, k * OF_CHUNK:(k + 1) * OF_CHUNK],
                            start=(k == 0), stop=(k == N_K - 1),
                        )
                    ot = op.tile(
                        [128, OF_CHUNK], dt.float32, tag="ot", name=f"ot{c}_{t}"
                    )
                    nc.scalar.activation(
                        ot[:], ps[:], AF.Copy, scale=dq[:, t:t + 1]
                    )
                    nc.gpsimd.dma_start(
                        out[t * 128:(t + 1) * 128,
                            c * OF_CHUNK:(c + 1) * OF_CHUNK],
                        ot[:],
                    )
                twT_cur = twT_next

    nc.compile()
    return nc


def _get_module():
    if "nc" not in _CACHE:
        _CACHE["nc"] = _build_module()
    return _CACHE["nc"]


def _make_in_maps(x2, w2):
    w2T = np.ascontiguousarray(w2.T)  # [in, of]
    maps = []
    for c in range(N_CORES):
        wrotT = np.concatenate(
            [
                w2T[:, ((c + 1 + j) % N_SLAB) * OF_CHUNK:
                    (((c + 1 + j) % N_SLAB) + 1) * OF_CHUNK]
                for j in range(N_SLAB - 1)
            ],
            axis=1,
        )
        maps.append(
            {
                "xs": x2[c * TOK_PC:(c + 1) * TOK_PC],
                "wmy": np.ascontiguousarray(
                    w2[c * OF_CHUNK:(c + 1) * OF_CHUNK]
                ),
                "wmyT": np.ascontiguousarray(
                    w2T[:, c * OF_CHUNK:(c + 1) * OF_CHUNK]
                ),
                "wrotT": np.ascontiguousarray(wrotT),
            }
        )
    return maps


def kernel(x: np.ndarray, weight: np.ndarray) -> np.ndarray:
    from concourse.bass_utils import run_bass_kernel_spmd

    x = np.asarray(x, dtype=np.float32)
    weight = np.asarray(weight, dtype=np.float32)
    x2 = np.ascontiguousarray(x.reshape(TOK, D_IN))
    w2 = np.ascontiguousarray(weight)

    in_maps = _make_in_maps(x2, w2)
    nc = _get_module()
    res = run_bass_kernel_spmd(nc, in_maps, list(range(N_CORES)))
    out_full = np.empty((TOK, D_OUT), dtype=np.float32)
    for c in range(N_CORES):
        oc = res.results[c]["out"]
        for j in range(N_SLAB):
            s = (c + j) % N_SLAB
            out_full[c * TOK_PC:(c + 1) * TOK_PC,
                     s * OF_CHUNK:(s + 1) * OF_CHUNK] = \
                oc[:, j * OF_CHUNK:(j + 1) * OF_CHUNK]
    return out_full.reshape(B, S, D_OUT)


# revision 32
# speedup vs baseline: 1.1335x; 1.0111x over previous
"""BitLinear (per-token int8 activation quant + ternary weight quant + matmul)
as a Bass/Tile kernel on 8 Trainium2 NeuronCores.

Strategy (data-parallel tokens + 8-way-sharded |W| mean + rotated slabs):
  - x [4,2048,4096] -> [8192,4096]; each core quantizes and matmuls its own
    1024-token slab against the FULL weight; outputs concatenate on tokens.
  - mean(|W|) is sharded: core c reads only its own 512-of-row slice of W
    (8.4 MB), computes a partial |W| sum on the GPSIMD engine (keeping the
    Vector engine free for activation quant), and a 512-byte AllReduce(add)
    yields the global sum.
  - W is passed HOST-TRANSPOSED ([in, of] layout) so ternarization produces
    matmul-ready [contraction, of] tiles directly -- no SBUF-to-SBUF xbar
    transposes in the matmul phase (they contended with PE operand streams
    and serialized behind the collective's DMA-completion lane).
  - Each core walks the 8 output slabs in ROTATED order starting with its
    own slice (first chunks prefetched during the AllReduce wait),
    ternarizing one slab ahead of the PE, interleaved per token tile so the
    Vector FIFO never head-of-line blocks the PSUM-drain scales.
  - q = rint(x*s) (s = 127/max(|x|) per token) and tw in {-1,0,1} are exact
    in bf16 => the bf16 matmul with fp32 PSUM accumulation is EXACT integer
    arithmetic; per-token dequant scales applied on the PSUM->SBUF copy.
"""
import numpy as np
from contextlib import ExitStack

N_CORES = 8
B, S, D_IN, D_OUT = 4, 2048, 4096, 4096
TOK = B * S                  # 8192
TOK_PC = TOK // N_CORES      # 1024 tokens per core
N_TOK_TILES = TOK_PC // 128  # 8
N_K = D_IN // 128            # 32 contraction tiles
OF_CHUNK = 512
N_SLAB = D_OUT // OF_CHUNK   # 8
NCH = 8                      # ternarize chunks per slab: [128, 4, 512] each
KCH = N_K // NCH             # k-tiles per chunk (4)
EPS = 1e-5
MAGIC = float(np.float32(1.5 * 2 ** 23))   # fp32 round-to-nearest-even trick
MEAN_SCALE = float(np.float32(1.0 / (D_IN * D_OUT)))  # 2^-24, exact

_CACHE = {}


def _build_module():
    import concourse.bacc as bacc
    import concourse.tile as tile
    import concourse.mybir as mybir
    import concourse.bass_isa as bass_isa

    dt = mybir.dt
    AF = mybir.ActivationFunctionType
    AL = mybir.AluOpType
    AX = mybir.AxisListType

    nc = bacc.Bacc(
        "TRN2", target_bir_lowering=False, debug=False, num_devices=N_CORES
    )
    xs = nc.dram_tensor("xs", [TOK_PC, D_IN], dt.float32, kind="ExternalInput").ap()
    # full W as bf16 (host-cast): each core computes the FULL mean(|W|)
    # locally -- no collective, so no AllReduce rank-skew and no
    # transpose-behind-collective fence. bf16 rounding is unbiased; the
    # mean error (~3e-7 rel) is far inside the ternary-threshold budget.
    wb16 = nc.dram_tensor(
        "wb16", [D_OUT, D_IN], dt.bfloat16, kind="ExternalInput"
    ).ap()
    # own W slice, transposed: [in, of] for out-feature rows [512c, 512c+512)
    wmyT = nc.dram_tensor(
        "wmyT", [D_IN, OF_CHUNK], dt.float32, kind="ExternalInput"
    ).ap()
    # remaining 7 slabs, transposed + rotated: of-cols of slab c+1, ... (mod 8)
    wrotT = nc.dram_tensor(
        "wrotT", [D_IN, (N_SLAB - 1) * OF_CHUNK], dt.float32, kind="ExternalInput"
    ).ap()
    out = nc.dram_tensor("out", [TOK_PC, D_OUT], dt.float32, kind="ExternalOutput").ap()

    HD = D_IN // 2
    HK = N_K // 2
    CH_F = KCH * OF_CHUNK    # 2048 free elems per ternarize chunk

    def wT_chunk(src, j, col0):
        # [128, KCH, OF_CHUNK] view of rows [j*512,(j+1)*512), cols [col0,+512)
        return src[j * 512:(j + 1) * 512, col0:col0 + OF_CHUNK].rearrange(
            "(a p) f -> p a f", p=128
        )

    with tile.TileContext(nc) as tc, ExitStack() as ctx:
        stats = ctx.enter_context(tc.tile_pool(name="stats", bufs=1))
        qT_pool = ctx.enter_context(tc.tile_pool(name="qT", bufs=N_TOK_TILES))
        big = ctx.enter_context(tc.tile_pool(name="big", bufs=3))
        qb_pool = ctx.enter_context(tc.tile_pool(name="qbp", bufs=2))
        twTp = ctx.enter_context(tc.tile_pool(name="twT", bufs=2))
        op = ctx.enter_context(tc.tile_pool(name="op", bufs=3))
        pp = ctx.enter_context(tc.tile_pool(name="pp", bufs=8, space="PSUM"))

        amc = stats.tile([128, N_TOK_TILES], dt.float32, tag="amc")
        s_all = stats.tile([128, N_TOK_TILES], dt.float32, tag="s_all")
        dq = stats.tile([128, N_TOK_TILES], dt.float32, tag="dq")
        wme = stats.tile([128, 1], dt.float32, tag="wme")
        swt = stats.tile([128, 1], dt.float32, tag="swt")
        wp2 = stats.tile([128, 2 * D_OUT // 128], dt.float32, tag="wp2")
        wsum_sb = stats.tile([128, 1], dt.float32, tag="wsum_sb")
        gtot = stats.tile([128, 1], dt.float32, tag="gtot")
        tmp1 = stats.tile([128, 1], dt.float32, tag="tmp1")

        # ---- |W| sum over the FULL weight from the bf16 copy, 64 chunks
        # [128, 2048]. Reduces alternate between the Scalar engine
        # (activation Abs + accum_out) and Vector, so neither engine
        # becomes the prologue wall. ----
        NMC = 2 * D_OUT // 128  # 64
        with nc.named_scope("wmean"), tc.high_priority():
            for j in range(NMC):
                wt = big.tile([128, CH_F], dt.bfloat16, tag="big", name=f"wm{j}")
                eng = nc.scalar if j % 2 == 0 else nc.sync
                eng.dma_start(
                    wt[:],
                    wb16[(j // 2) * 128:(j // 2 + 1) * 128,
                         (j % 2) * CH_F:(j % 2 + 1) * CH_F],
                )
                if j % 2 == 0:
                    nc.scalar.activation(
                        wt[:], wt[:], AF.Abs, accum_out=wp2[:, j:j + 1]
                    )
                else:
                    nc.vector.tensor_reduce(
                        wp2[:, j:j + 1], wt[:], axis=AX.X, op=AL.add,
                        apply_absolute_value=True,
                    )
            nc.vector.tensor_reduce(wsum_sb[:], wp2[:], axis=AX.X, op=AL.add)

        # ---- x-quant: own tokens -> resident qT tiles (half tiles) ----
        # software-pipelined: tile t's scale/round/transpose are emitted
        # after tile t+1's amax chain, so the Vector FIFO never stalls
        # waiting for the Scalar engine's scale pass. NOTE: the qT
        # DMA-transposes unconditionally serialize behind the in-flight
        # AllReduce (runtime behavior, independent of schedule priority),
        # so they burst once the mesh completes (~65-130 us); the matmul
        # ramp absorbs most of that.
        qT_tiles = []
        with nc.named_scope("xquant"), tc.tile_pool(name="xq", bufs=4) as xq:

            def xq_flush(t, xh, qT_t):
                for h in range(2):
                    nc.scalar.activation(
                        xh[h][:], xh[h][:], AF.Copy, scale=s_all[:, t:t + 1]
                    )
                    qbh = qb_pool.tile(
                        [128, HD], dt.bfloat16, tag="qb", name=f"qb{t}_{h}"
                    )
                    nc.vector.tensor_scalar(
                        qbh[:], xh[h][:], MAGIC, MAGIC, op0=AL.add, op1=AL.subtract
                    )
                    nc.sync.dma_start(
                        qT_t[:, h * HK:(h + 1) * HK, :], qbh[:], transpose=True
                    )

            pend = None
            for t in range(N_TOK_TILES):
                qT_t = qT_pool.tile(
                    [128, N_K, 128], dt.bfloat16, tag="qT", name=f"qT{t}"
                )
                xh = []
                for h in range(2):
                    xth = xq.tile([128, HD], dt.float32, tag="xq", name=f"xt{t}_{h}")
                    eng = nc.sync if h == 0 else nc.scalar
                    eng.dma_start(
                        xth[:], xs[t * 128:(t + 1) * 128, h * HD:(h + 1) * HD]
                    )
                    nc.vector.tensor_reduce(
                        amc[:, t:t + 1] if h == 0 else tmp1[:],
                        xth[:], axis=AX.X, op=AL.max, apply_absolute_value=True,
                    )
                    xh.append(xth)
                # amax = max(half0, half1); then clip, s = 127/amax_c
                nc.vector.tensor_tensor(
                    amc[:, t:t + 1], amc[:, t:t + 1], tmp1[:], op=AL.max
                )
                nc.vector.tensor_scalar(
                    amc[:, t:t + 1], amc[:, t:t + 1], EPS, None, op0=AL.max
                )
                nc.vector.reciprocal(s_all[:, t:t + 1], amc[:, t:t + 1])
                nc.vector.tensor_scalar(
                    s_all[:, t:t + 1], s_all[:, t:t + 1], 127.0, None, op0=AL.mult
                )
                if pend is not None:
                    xq_flush(*pend)
                pend = (t, xh, qT_t)
                qT_tiles.append(qT_t)
            xq_flush(*pend)

        # ---- prefetch own-slab chunks during the AllReduce wait (scalar/
        # sync queues; gpsimd stays clear so the AR chain fires asap) ----
        own_pref = []
        for m in range(4):
            wt = big.tile([128, CH_F], dt.float32, tag="big", name=f"wo{m}")
            eng = nc.scalar if m % 2 == 0 else nc.sync
            eng.dma_start(wt[:], wT_chunk(wmyT, m, 0))
            own_pref.append(wt)

        # ---- cross-partition reduce of the per-partition |W| sums (local
        # only; every core holds the full sum -- no collective) ----
        with nc.named_scope("ar"):
            with tc.high_priority():
                nc.gpsimd.partition_all_reduce(
                    gtot[:], wsum_sb[:], channels=128,
                    reduce_op=bass_isa.ReduceOp.add,
                )
            nc.vector.tensor_scalar(
                wme[:], gtot[:], MEAN_SCALE, EPS, op0=AL.mult, op1=AL.max
            )
            nc.vector.reciprocal(swt[:], wme[:])
            for t in range(N_TOK_TILES):
                nc.vector.tensor_scalar(
                    dq[:, t:t + 1], amc[:, t:t + 1], wme[:, 0:1],
                    float(np.float32(1.0 / 127.0)), op0=AL.mult, op1=AL.mult,
                )

        # ---- per-slab ternarize, chunk [128, 2048] = 4 k-tiles x 512 of.
        # W arrives pre-transposed, so round/clamp write matmul-ready tiles;
        # no transposes. Round to a twT slice, clamp in place. ----
        def tern_chunk(wt, twT_c, j, tag):
            nc.scalar.activation(wt[:], wt[:], AF.Copy, scale=swt[:, 0:1])
            dst = twT_c[:, j * CH_F:(j + 1) * CH_F]
            nc.vector.tensor_scalar(
                dst, wt[:], MAGIC, MAGIC, op0=AL.add, op1=AL.subtract
            )
            nc.vector.tensor_scalar(
                dst, dst, 1.0, -1.0, op0=AL.min, op1=AL.max
            )

        def tern_own():
            twT_c = twTp.tile(
                [128, N_K * OF_CHUNK], dt.bfloat16, tag="twT", name="twT_own"
            )
            for m in range(NCH):
                if m < len(own_pref):
                    wt = own_pref[m]
                else:
                    wt = big.tile([128, CH_F], dt.float32, tag="big", name=f"wo{m}")
                    eng = nc.scalar if m % 2 == 0 else nc.sync
                    eng.dma_start(wt[:], wT_chunk(wmyT, m, 0))
                tern_chunk(wt, twT_c, m, f"o{m}")
            return twT_c

        # next slab's ternarize is emitted interleaved with this slab's
        # token tiles so the Vector FIFO alternates tern ops with PSUM-drain
        # scales (psum banks free promptly; no head-of-line stall).
        def make_tern_stream(c):
            twT_c = twTp.tile(
                [128, N_K * OF_CHUNK], dt.bfloat16, tag="twT", name=f"twT{c}"
            )
            col0 = (c - 1) * OF_CHUNK
            wts = {}

            def load(m):
                wt = big.tile([128, CH_F], dt.float32, tag="big", name=f"wt{c}_{m}")
                eng = nc.scalar if m % 2 == 0 else nc.sync
                eng.dma_start(wt[:], wT_chunk(wrotT, m, col0))
                wts[m] = wt

            def compute(m):
                tern_chunk(wts.pop(m), twT_c, m, f"{c}_{m}")

            return twT_c, load, compute

        with nc.named_scope("mm"):
            twT_cur = tern_own()
            for c in range(N_SLAB):
                if c + 1 < N_SLAB:
                    twT_next, tern_load, tern_compute = make_tern_stream(c + 1)
                    tern_load(0)
                    tern_load(1)
                else:
                    twT_next = tern_load = tern_compute = None
                for t in range(N_TOK_TILES):
                    if tern_load is not None:
                        if t + 2 < NCH:
                            tern_load(t + 2)
                        tern_compute(t)
                    ps = pp.tile(
                        [128, OF_CHUNK], dt.float32, tag="ps", name=f"ps{c}_{t}"
                    )
                    for k in range(N_K):
                        nc.tensor.matmul(
                            ps[:], qT_tiles[t][:, k, :],
                            twT_cur[:, k * OF_CHUNK:(k + 1) * OF_CHUNK],
                            start=(k == 0), stop=(k == N_K - 1),
                        )
                    ot = op.tile(
                        [128, OF_CHUNK], dt.float32, tag="ot", name=f"ot{c}_{t}"
                    )
                    nc.scalar.activation(
                        ot[:], ps[:], AF.Copy, scale=dq[:, t:t + 1]
                    )
                    nc.gpsimd.dma_start(
                        out[t * 128:(t + 1) * 128,
                            c * OF_CHUNK:(c + 1) * OF_CHUNK],
                        ot[:],
                    )
                twT_cur = twT_next

    nc.compile()
    return nc


def _get_module():
    if "nc" not in _CACHE:
        _CACHE["nc"] = _build_module()
    return _CACHE["nc"]


def _make_in_maps(x2, w2):
    import ml_dtypes

    w2T = np.ascontiguousarray(w2.T)  # [in, of]
    wb16 = np.ascontiguousarray(w2.astype(ml_dtypes.bfloat16))
    maps = []
    for c in range(N_CORES):
        wrotT = np.concatenate(
            [
                w2T[:, ((c + 1 + j) % N_SLAB) * OF_CHUNK:
                    (((c + 1 + j) % N_SLAB) + 1) * OF_CHUNK]
                for j in range(N_SLAB - 1)
            ],
            axis=1,
        )
        maps.append(
            {
                "xs": x2[c * TOK_PC:(c + 1) * TOK_PC],
                "wb16": wb16,
                "wmyT": np.ascontiguousarray(
                    w2T[:, c * OF_CHUNK:(c + 1) * OF_CHUNK]
                ),
                "wrotT": np.ascontiguousarray(wrotT),
            }
        )
    return maps


def kernel(x: np.ndarray, weight: np.ndarray) -> np.ndarray:
    from concourse.bass_utils import run_bass_kernel_spmd

    x = np.asarray(x, dtype=np.float32)
    weight = np.asarray(weight, dtype=np.float32)
    x2 = np.ascontiguousarray(x.reshape(TOK, D_IN))
    w2 = np.ascontiguousarray(weight)

    in_maps = _make_in_maps(x2, w2)
    nc = _get_module()
    res = run_bass_kernel_spmd(nc, in_maps, list(range(N_CORES)))
    out_full = np.empty((TOK, D_OUT), dtype=np.float32)
    for c in range(N_CORES):
        oc = res.results[c]["out"]
        for j in range(N_SLAB):
            s = (c + j) % N_SLAB
            out_full[c * TOK_PC:(c + 1) * TOK_PC,
                     s * OF_CHUNK:(s + 1) * OF_CHUNK] = \
                oc[:, j * OF_CHUNK:(j + 1) * OF_CHUNK]
    return out_full.reshape(B, S, D_OUT)
